# revision 22
# baseline (speedup 1.0000x reference)
"""Trainium2 Bass kernel for nn_End2EndRVFixedOutput (nms_detection).

Reference semantics: out[100,7] starts at zeros; for n = 0..7 in order,
with off_n = (0 if n==0 else num_dets[n-1]) and k_n = num_dets[n],
rows [off_n, off_n+k_n) are overwritten with
[n, boxes[n,j,0:4], classes[n,j], scores[n,j]] for j = row-off_n.

num_dets < 12, so only the [:, :12] input slices matter and only out rows
0..21 can ever be written.  Device algorithm (per core, inputs replicated):

  1. x7[96,7] = [vd | boxes | classes | scores] for rows p = 12n+j is
     assembled by direct column DMAs straight from the full DRAM tensors.
  2. Winner masks are computed deterministically on DVE + PE:
        rm8[n,r]  = (off_n <= r < off_n+k_n)        # batch n covers row r
        stn[n,r]  = sum_{m>n} rm8[m,r]              # tiny suffix matmul
        effT[n,r] = rm8[n,r] * (stn[n,r]==0)        # n is the LAST writer
        EFF96     = SEL96 @ effT                    # broadcast to (n,j) rows
     (both matmuls in bf16 -- operands are exact small ints -- single pass)
     Per-row scatter targets and winner gating:
        rpv[p]  = off_n + j + 1e6*(j >= k_n)
        w96[p]  = sum_r (R100[p,r]==rpv[p]) * EFF96[p,r]   # fused accum_out
        ridx[p] = rpv[p] + 1e6*(1 - w96[p])
  3. A zero-fill indirect DMA writes zeros to all 100 rows, then the data
     scatter writes x7 rows to out[ridx] on the same qPoolDynamic ring.
     Gating makes destinations UNIQUE (at most one winner per row), so no
     reliance on DMA descriptor ordering; indices >= 1e6 are skipped via
     bounds_check.

All arithmetic is exact (masks are 0/1, indices are small ints), so the
output matches the reference bit-for-bit.  Every core runs the full
(tiny) computation; core 0's output is returned.
"""

import sys

import numpy as np

_TRN_REPO = "/opt/trn_rl_repo"
if _TRN_REPO not in sys.path:
    sys.path.insert(0, _TRN_REPO)

import ml_dtypes

import concourse.bacc as bacc
import concourse.bass as bass
import concourse.mybir as mybir
import concourse.tile as tile
from concourse.bass_types import AP
from concourse.bass_utils import run_bass_kernel_spmd

B = 8          # batches
N_FULL = 8192  # detections per batch in the full input
J = 12         # num_dets < 12, so only rows [:12] of each batch matter
R = 100        # fixed output rows
P96 = B * J    # 96 stacked (batch, j) rows
OOB = 1.0e6    # pushed past bounds_check so the scatter skips the row

F32 = mybir.dt.float32
BF16 = mybir.dt.bfloat16
I32 = mybir.dt.int32

# f32 constant blob: CB96 [96,102] = R100 | j96 | vd96
CONST_LEN = P96 * (R + 2)
# bf16 constant blob: U96 [8,96] | SEL96 [8,96] packed per-row as [8,192]
CONSTBF_LEN = 8 * (2 * P96)


def _make_consts():
    p = np.arange(P96)
    m = np.arange(B)
    r100 = np.tile(np.arange(R, dtype=np.float32)[None, :], (P96, 1))    # [96,100]
    j96 = (p % J).astype(np.float32)[:, None]                            # [96,1]
    vd96 = (p // J).astype(np.float32)[:, None]                          # [96,1]
    blob = np.concatenate([r100, j96, vd96], axis=1).ravel().astype(np.float32)
    assert blob.shape == (CONST_LEN,)
    u96 = (m[:, None] > p[None, :] // J).astype(np.float32)              # [8,96]
    sel96 = (m[:, None] == p[None, :] // J).astype(np.float32)           # [8,96]
    blobbf = (
        np.concatenate([u96, sel96], axis=1).ravel().astype(ml_dtypes.bfloat16)
    )
    assert blobbf.shape == (CONSTBF_LEN,)
    return np.ascontiguousarray(blob), np.ascontiguousarray(blobbf)


def _build_nc() -> bass.Bass:
    nc = bacc.Bacc(None, target_bir_lowering=False)
    nd_d = nc.dram_tensor("num_dets", [B], I32, kind="ExternalInput")
    boxes_d = nc.dram_tensor("boxes", [B, N_FULL, 4], F32, kind="ExternalInput")
    scores_d = nc.dram_tensor("scores", [B, N_FULL], F32, kind="ExternalInput")
    classes_d = nc.dram_tensor("classes", [B, N_FULL], F32, kind="ExternalInput")
    const_d = nc.dram_tensor("consts", [CONST_LEN], F32, kind="ExternalInput")
    constbf_d = nc.dram_tensor("constsbf", [CONSTBF_LEN], BF16, kind="ExternalInput")
    out_d = nc.dram_tensor("out", [R, 7], F32, kind="ExternalOutput")

    with tile.TileContext(nc) as tc:
        with (
            tc.tile_pool(name="sb", bufs=1) as sb,
            tc.tile_pool(name="ps", bufs=1, space=bass.MemorySpace.PSUM) as ps,
        ):
            ndi = sb.tile([B, 1], I32)
            k96 = sb.tile([P96, 1], I32)
            off96 = sb.tile([P96, 1], I32)
            cb96 = sb.tile([P96, R + 2], F32)
            usel = sb.tile([B, 2 * P96], BF16)
            x7 = sb.tile([P96, 7], F32)
            z7 = sb.tile([R, 7], F32)
            ridx0 = sb.tile([R, 1], I32)

            k32 = sb.tile([32, 1], F32)
            off32 = sb.tile([32, 1], F32)
            s8f = sb.tile([B, 1], F32)
            u8c = sb.tile([B, R], F32)
            rm8 = sb.tile([B, R], BF16)
            b2 = sb.tile([P96, 1], F32)
            rpv = sb.tile([P96, 1], F32)
            scr96 = sb.tile([P96, R], F32)
            scr96b = sb.tile([P96, R], F32)
            q96 = sb.tile([P96, 1], F32)
            g96 = sb.tile([P96, 1], F32)
            w96 = sb.tile([P96, 1], F32)
            c96 = sb.tile([P96, 1], F32)
            ridx = sb.tile([P96, 1], I32)

            stn96 = ps.tile([P96, R], F32)
            rm96 = ps.tile([P96, R], F32)

            U96 = usel[:, 0:P96]
            SEL96 = usel[:, P96 : 2 * P96]
            R100 = cb96[:, 0:R]
            R8 = cb96[0:B, 0:R]
            J96 = cb96[:, R : R + 1]
            VD96 = cb96[:, R + 1 : R + 2]

            # repeat-APs over num_dets: k96[12n+j] = nd[n]; off96[12n+j] = nd[n-1]
            nd_rep8 = AP(nd_d, 0, [[1, B], [0, J], [1, 1]])
            nd_rep7 = AP(nd_d, 0, [[1, B - 1], [0, J], [1, 1]])

            nc.gpsimd.memset(off96[:], 0)
            nc.gpsimd.memset(k32[:], 0.0)
            nc.gpsimd.memset(z7[:], 0.0)
            nc.gpsimd.iota(ridx0[:], pattern=[[1, 1]], base=0, channel_multiplier=1)

            # zero-fill pass: scatter zeros to every out row, on the same
            # qPoolDynamic ring as the data scatter, so skipped rows are zero
            zfill = nc.gpsimd.indirect_dma_start(
                out=out_d[:],
                out_offset=bass.IndirectOffsetOnAxis(ap=ridx0[:], axis=0),
                in_=z7[:],
                in_offset=None,
                bounds_check=R - 1,
                oob_is_err=False,
            )

            # loads on the two HWDGE queues only (gpsimd is kept free for the
            # indirect scatters); critical ones first
            nc.sync.dma_start(out=ndi[:], in_=nd_d[:].rearrange("(p f) -> p f", f=1))
            nc.sync.dma_start(out=k96[:], in_=nd_rep8)
            nc.sync.dma_start(out=off96[J:P96, :], in_=nd_rep7)
            nc.sync.dma_start(out=x7[:, 5:6], in_=classes_d[:, 0:J])
            nc.scalar.dma_start(
                out=cb96[:], in_=const_d[:].rearrange("(p f) -> p f", p=P96)
            )
            nc.scalar.dma_start(out=usel[:], in_=constbf_d[:].rearrange(
                "(p f) -> p f", p=B
            ))
            nc.scalar.dma_start(out=x7[:, 1:5], in_=boxes_d[:, 0:J, :])
            nc.scalar.dma_start(out=x7[:, 6:7], in_=scores_d[:, 0:J])

            alu = mybir.AluOpType
            vec = nc.vector

            # k32[0:8] = float(num_dets); off32[n] = k32[n-1] via partition shift
            vec.tensor_copy(k32[0:B, :], ndi[:])
            vec.stream_shuffle(off32[:], k32[:], mask=[31] + list(range(31)))
            # vd column of x7 comes straight out of the const tile
            vec.tensor_copy(x7[:, 0:1], VD96)
            # batch coverage masks on 8 partitions
            vec.tensor_tensor(s8f[:], k32[0:B, :], off32[0:B, :], alu.add)
            vec.tensor_scalar(u8c[:], R8, off32[0:B, :], None, alu.is_ge)
            vec.scalar_tensor_tensor(
                rm8[:], R8, s8f[:], u8c[:], alu.is_lt, alu.mult
            )
            # two parallel matmuls broadcast coverage + suffix-coverage to the
            # 96 (n,j) rows: stn96[p,r] = sum_{m>n} rm8[m,r], rm96[p,r] = rm8[n,r]
            nc.tensor.matmul(stn96[:], U96, rm8[:], start=True, stop=True)
            nc.tensor.matmul(rm96[:], SEL96, rm8[:], start=True, stop=True)

            # per-(n,j) scatter targets (fills DVE gaps while PE runs)
            vec.tensor_scalar(b2[:], k96[:], J96, OOB, alu.is_le, alu.mult)
            vec.scalar_tensor_tensor(
                rpv[:], off96[:], J96, b2[:], alu.add, alu.add
            )

            # one-hot column extraction at r = rpv[p] (single PSUM input each):
            #   q96[p] = stn96[p, rpv[p]]   g96[p] = rm96[p, rpv[p]]
            vec.scalar_tensor_tensor(
                scr96[:], R100, rpv[:], stn96[:], alu.is_equal, alu.mult,
                accum_out=q96[:],
            )
            vec.scalar_tensor_tensor(
                scr96b[:], R100, rpv[:], rm96[:], alu.is_equal, alu.mult,
                accum_out=g96[:],
            )
            # winner iff batch n covers its row and no later batch covers it
            vec.scalar_tensor_tensor(
                w96[:], q96[:], 0.0, g96[:], alu.is_equal, alu.mult
            )
            vec.tensor_scalar(c96[:], w96[:], -OOB, OOB, alu.mult, alu.add)
            vec.tensor_tensor(ridx[:], rpv[:], c96[:], alu.add)

            # winner-only scatter: destinations are unique, no ordering needed
            scat = nc.gpsimd.indirect_dma_start(
                out=out_d[:],
                out_offset=bass.IndirectOffsetOnAxis(ap=ridx[:], axis=0),
                in_=x7[:],
                in_offset=None,
                bounds_check=R - 1,
                oob_is_err=False,
            )
            # the zero-fill must fully land before the data scatter
            bass._add_dep_helper(
                scat.ins, zfill.ins, sync=True, reason="zero-fill before scatter"
            )

    nc.finalize()
    return nc


_CACHE: dict = {}


def _get_built():
    if "nc" not in _CACHE:
        _CACHE["nc"] = _build_nc()
        _CACHE["consts"] = _make_consts()
    return _CACHE["nc"], _CACHE["consts"]


def run(inputs: dict, trace: bool = False, **spmd_kwargs):
    """Run on all 8 cores with replicated inputs; returns (out, BassKernelResults)."""
    nc, (consts, constsbf) = _get_built()
    in_map = {
        "num_dets": np.ascontiguousarray(inputs["num_dets"], dtype=np.int32),
        "boxes": np.ascontiguousarray(inputs["boxes"], dtype=np.float32),
        "scores": np.ascontiguousarray(inputs["scores"], dtype=np.float32),
        "classes": np.ascontiguousarray(inputs["classes"], dtype=np.float32),
        "consts": consts,
        "constsbf": constsbf,
    }
    res = run_bass_kernel_spmd(
        nc,
        [dict(in_map) for _ in range(8)],
        core_ids=list(range(8)),
        trace=trace,
        **spmd_kwargs,
    )
    return res.results[0]["out"], res


def kernel(num_dets, boxes, scores, classes):
    out, _ = run(
        {"num_dets": num_dets, "boxes": boxes, "scores": scores, "classes": classes}
    )
    return out
